# revision 6
# baseline (speedup 1.0000x reference)
"""AttnBlock (GroupNorm + single-head 1x1-conv attention + residual) on 8 TRN2 cores.

Data-parallel over batch: core i processes x[i] (512, 64*64) entirely on-chip.

Math (per batch item, N = 64*64 = 4096 spatial positions, C = 512 channels):
  R = groupnorm(x)                          [C, N]
  scores = (Wq R)^T (Wk R) / sqrt(C) = R^T Ws R / sqrt(C),  Ws = Wq^T Wk (host)
  attn   = softmax(scores, axis=m)  (no max subtraction -- scores are O(1))
  out    = x + Wp (V attn^T) / d + biases,  V = Wk R + kb
All big matmuls run in bf16 on the PE at 1 col/cycle; accumulation fp32 in PSUM.
Softmax denominator d[n] is accumulated with a ones-matmul so everything stays
in the transposed [m, n] layout and no on-chip transposes are needed.
"""
import sys

sys.path.insert(0, "/opt/trn_rl_repo")

import numpy as np
import ml_dtypes

import concourse.bass as bass
import concourse.bacc as bacc
import concourse.mybir as mybir
import concourse.tile as tile
from concourse import bass_utils

F32 = mybir.dt.float32
BF16 = mybir.dt.bfloat16
AF = mybir.ActivationFunctionType
OP = mybir.AluOpType

B = 8
C = 512
N = 4096          # 64*64 spatial
GROUPS = 32
GSIZE = 16        # channels per group
EPS = 1e-6
CCH = 4           # channel chunks of 128
NCH = 8           # n chunks of 512
MT = 32           # m tiles of 128
P = 128
NW = 512          # matmul free dim / n-chunk width
INV_SQRT_C = 1.0 / float(np.sqrt(C))

_BUILD_CACHE = {}


def _build(use_amt: bool, use_kb: bool):
    nc = bacc.Bacc("TRN2", target_bir_lowering=False)

    x_in = nc.dram_tensor("x_in", [C, N], F32, kind="ExternalInput")
    wst_d = nc.dram_tensor("wst", [C, C], BF16, kind="ExternalInput")
    kwt_d = nc.dram_tensor("kwt", [C, C], BF16, kind="ExternalInput")
    pwt_d = nc.dram_tensor("pwt", [C, C], BF16, kind="ExternalInput")
    gamma_d = nc.dram_tensor("gamma_r", [P, CCH], F32, kind="ExternalInput")
    beta_d = nc.dram_tensor("beta_r", [P, CCH], F32, kind="ExternalInput")
    pb_d = nc.dram_tensor("pb_r", [P, CCH], F32, kind="ExternalInput")
    ones_d = nc.dram_tensor("ones_b", [P, P], BF16, kind="ExternalInput")
    g_d = nc.dram_tensor("gmat", [P, 8], F32, kind="ExternalInput")
    g2_d = nc.dram_tensor("g2mat", [8, P], F32, kind="ExternalInput")
    if use_kb:
        kbb_d = nc.dram_tensor("kbb", [P, NW], F32, kind="ExternalInput")
    if use_amt:
        qbb_d = nc.dram_tensor("qbb", [P, NW], F32, kind="ExternalInput")
    out_d = nc.dram_tensor("out", [C, N], F32, kind="ExternalOutput")

    with tile.TileContext(nc) as tc:
        # ---- persistent pools (allocated bottom of SBUF stack) ----
        const = tc.alloc_tile_pool(name="const", bufs=1)
        r_pool = tc.alloc_tile_pool(name="r_pool", bufs=CCH)
        u_pool = tc.alloc_tile_pool(name="u_pool", bufs=CCH)
        vt_pool = tc.alloc_tile_pool(name="vt_pool", bufs=MT)
        et_pool = tc.alloc_tile_pool(name="et_pool", bufs=8)
        osb_pool = tc.alloc_tile_pool(name="osb_pool", bufs=8)
        rd_pool = tc.alloc_tile_pool(name="rd_pool", bufs=2)
        tmp_pool = tc.alloc_tile_pool(name="tmp_pool", bufs=3)
        xres_pool = tc.alloc_tile_pool(name="xres_pool", bufs=3)
        outsb_pool = tc.alloc_tile_pool(name="outsb_pool", bufs=3)

        wst_sb = const.tile([P, CCH, NW], BF16)
        kwt_sb = const.tile([P, CCH, NW], BF16)
        pwt_sb = const.tile([P, CCH, NW], BF16)
        gamma_sb = const.tile([P, CCH], F32)
        beta_sb = const.tile([P, CCH], F32)
        pb_sb = const.tile([P, CCH], F32)
        ones_sb = const.tile([P, P], BF16)
        g_sb = const.tile([P, 8], F32)
        g2_sb = const.tile([8, P], F32)
        for cp in range(CCH):
            nc.sync.dma_start(out=wst_sb[:, cp, :], in_=wst_d[cp * P:(cp + 1) * P, :])
            nc.sync.dma_start(out=kwt_sb[:, cp, :], in_=kwt_d[cp * P:(cp + 1) * P, :])
            nc.sync.dma_start(out=pwt_sb[:, cp, :], in_=pwt_d[cp * P:(cp + 1) * P, :])
        nc.sync.dma_start(out=gamma_sb, in_=gamma_d[:, :])
        nc.sync.dma_start(out=beta_sb, in_=beta_d[:, :])
        nc.sync.dma_start(out=pb_sb, in_=pb_d[:, :])
        nc.sync.dma_start(out=ones_sb, in_=ones_d[:, :])
        nc.sync.dma_start(out=g_sb, in_=g_d[:, :])
        nc.sync.dma_start(out=g2_sb, in_=g2_d[:, :])
        if use_kb:
            kbb_sb = const.tile([P, NW], F32)
            nc.sync.dma_start(out=kbb_sb, in_=kbb_d[:, :])
        if use_amt:
            qbb_sb = const.tile([P, NW], F32)
            nc.sync.dma_start(out=qbb_sb, in_=qbb_d[:, :])
            amt_sb = const.tile([P, MT], F32)
            ascr_sb = const.tile([P, NW], F32)

        r_sb = []

        # ================= stage 1: GroupNorm =================
        with tc.tile_pool(name="x1_pool", bufs=2) as x1_pool, \
             tc.tile_pool(name="bn_pool", bufs=2) as bn_pool, \
             tc.tile_pool(name="st_pool", bufs=4) as st_pool, \
             tc.tile_pool(name="ab_pool", bufs=4) as ab_pool, \
             tc.tile_pool(name="pstat", bufs=2, space="PSUM") as pstat:
            for cp in range(CCH):
                x1 = x1_pool.tile([P, N], F32, tag="x1")
                nc.sync.dma_start(out=x1, in_=x_in[cp * P:(cp + 1) * P, :])
                bnst = bn_pool.tile([P, 8, 6], F32, tag="bnst")
                for s in range(8):
                    nc.vector.bn_stats(out=bnst[:, s, :], in_=x1[:, s * NW:(s + 1) * NW])
                mv = bn_pool.tile([P, 2], F32, tag="mv")
                nc.vector.bn_aggr(out=mv, in_=bnst)
                # S: per-partition [mean, E[x^2]]
                s_sb = bn_pool.tile([P, 2], F32, tag="s_sb")
                nc.vector.tensor_copy(out=s_sb[:, 0:1], in_=mv[:, 0:1])
                nc.vector.scalar_tensor_tensor(
                    out=s_sb[:, 1:2], in0=mv[:, 0:1], scalar=mv[:, 0:1],
                    in1=mv[:, 1:2], op0=OP.mult, op1=OP.add)
                # group-aggregate across the 16-partition groups via PE
                psg = pstat.tile([8, 2], F32, tag="psg")
                nc.tensor.matmul(psg, lhsT=g_sb, rhs=s_sb, start=True, stop=True)
                mu = st_pool.tile([8, 1], F32, tag="mu")
                nc.vector.tensor_scalar_mul(out=mu, in0=psg[:, 0:1], scalar1=1.0 / GSIZE)
                ex2 = st_pool.tile([8, 1], F32, tag="ex2")
                nc.vector.tensor_scalar_mul(out=ex2, in0=psg[:, 1:2], scalar1=1.0 / GSIZE)
                musq = st_pool.tile([8, 1], F32, tag="musq")
                nc.vector.tensor_mul(out=musq, in0=mu, in1=mu)
                veps = st_pool.tile([8, 1], F32, tag="veps")
                nc.vector.scalar_tensor_tensor(
                    out=veps, in0=ex2, scalar=EPS, in1=musq, op0=OP.add, op1=OP.subtract)
                sd = st_pool.tile([8, 1], F32, tag="sd")
                nc.scalar.activation(out=sd, in_=veps, func=AF.Sqrt)
                rs0 = st_pool.tile([8, 1], F32, tag="rs0")
                nc.vector.reciprocal(out=rs0, in_=sd)
                # one Newton step for rsqrt accuracy: rs1 = rs0*(1.5 - 0.5*veps*rs0^2)
                t1 = st_pool.tile([8, 1], F32, tag="t1")
                nc.vector.tensor_mul(out=t1, in0=rs0, in1=rs0)
                t2 = st_pool.tile([8, 1], F32, tag="t2")
                nc.vector.tensor_mul(out=t2, in0=t1, in1=veps)
                t3 = st_pool.tile([8, 1], F32, tag="t3")
                nc.vector.tensor_scalar(
                    out=t3, in0=t2, scalar1=-0.5, scalar2=1.5, op0=OP.mult, op1=OP.add)
                rs1 = st_pool.tile([8, 1], F32, tag="rs1")
                nc.vector.tensor_mul(out=rs1, in0=t3, in1=rs0)
                w_sb = st_pool.tile([8, 2], F32, tag="w_sb")
                nc.vector.tensor_copy(out=w_sb[:, 0:1], in_=rs1)
                nc.vector.tensor_copy(out=w_sb[:, 1:2], in_=mu)
                # broadcast back to per-partition values via PE
                psp2 = pstat.tile([P, 2], F32, tag="psp2")
                nc.tensor.matmul(psp2, lhsT=g2_sb, rhs=w_sb, start=True, stop=True)
                a_c = ab_pool.tile([P, 1], F32, tag="a_c")
                nc.vector.tensor_mul(out=a_c, in0=gamma_sb[:, cp:cp + 1], in1=psp2[:, 0:1])
                tb = st_pool.tile([P, 1], F32, tag="tb")
                nc.vector.tensor_mul(out=tb, in0=psp2[:, 1:2], in1=a_c)
                b_c = ab_pool.tile([P, 1], F32, tag="b_c")
                nc.vector.tensor_sub(out=b_c, in0=beta_sb[:, cp:cp + 1], in1=tb)
                r_c = r_pool.tile([P, N], BF16, tag="r")
                nc.scalar.activation(out=r_c, in_=x1, func=AF.Identity, bias=b_c, scale=a_c)
                r_sb.append(r_c)

        # ================= stage 2: U = Ws R, VT = (Wk R + kb)^T =================
        u_sb = []
        vt_sb = []
        with tc.tile_pool(name="psv", bufs=4, space="PSUM") as psv_pool:
            for cq in range(CCH):
                u_c = u_pool.tile([P, N], BF16, tag="u")
                for mc in range(NCH):
                    psv = psv_pool.tile([P, NW], F32, tag="psv")
                    for cb in range(CCH):
                        nc.tensor.matmul(
                            psv,
                            lhsT=wst_sb[:, cb, cq * P:(cq + 1) * P],
                            rhs=r_sb[cb][:, mc * NW:(mc + 1) * NW],
                            start=(cb == 0), stop=(cb == CCH - 1))
                    nc.scalar.activation(
                        out=u_c[:, mc * NW:(mc + 1) * NW], in_=psv, func=AF.Copy)
                u_sb.append(u_c)
            for mt in range(MT):
                vt_t = vt_pool.tile([P, NW], BF16, tag="vt")
                psv = psv_pool.tile([P, NW], F32, tag="psv")
                for cb in range(CCH):
                    nc.tensor.matmul(
                        psv,
                        lhsT=r_sb[cb][:, mt * P:(mt + 1) * P],
                        rhs=kwt_sb[:, cb, :],
                        start=(cb == 0), stop=(cb == CCH - 1))
                if use_kb:
                    nc.vector.tensor_add(out=vt_t, in0=psv, in1=kbb_sb)
                else:
                    nc.scalar.activation(out=vt_t, in_=psv, func=AF.Copy)
                if use_amt:
                    nc.vector.scalar_tensor_tensor(
                        out=ascr_sb, in0=vt_t, scalar=INV_SQRT_C, in1=qbb_sb,
                        op0=OP.mult, op1=OP.mult, accum_out=amt_sb[:, mt:mt + 1])
                vt_sb.append(vt_t)

        # ================= stage 3: attention + proj + residual =================
        with tc.tile_pool(name="pss", bufs=2, space="PSUM") as pss_pool, \
             tc.tile_pool(name="pso", bufs=1, space="PSUM") as pso_pool, \
             tc.tile_pool(name="psd", bufs=1, space="PSUM") as psd_pool, \
             tc.tile_pool(name="psp", bufs=1, space="PSUM") as psp_pool:

            def emit_dpv(et_t, mt, psd_t, pso_tiles, first, last):
                nc.tensor.matmul(psd_t, lhsT=ones_sb, rhs=et_t, start=first, stop=last)
                for cs in range(CCH):
                    nc.tensor.matmul(
                        pso_tiles[cs],
                        lhsT=vt_sb[mt][:, cs * P:(cs + 1) * P],
                        rhs=et_t, start=first, stop=last)

            def emit_proj(state, oc):
                osb_list, rd_t, pnch = state
                psp = psp_pool.tile([P, NW], F32, tag="psp")
                for cs in range(CCH):
                    nc.tensor.matmul(
                        psp,
                        lhsT=pwt_sb[:, cs, oc * P:(oc + 1) * P],
                        rhs=osb_list[cs],
                        start=(cs == 0), stop=(cs == CCH - 1))
                xr = xres_pool.tile([P, NW], F32, tag="xr")
                nc.sync.dma_start(
                    out=xr, in_=x_in[oc * P:(oc + 1) * P, pnch * NW:(pnch + 1) * NW])
                t_t = tmp_pool.tile([P, NW], F32, tag="t_t")
                nc.vector.tensor_mul(out=t_t, in0=psp, in1=rd_t)
                ob = outsb_pool.tile([P, NW], F32, tag="ob")
                nc.vector.scalar_tensor_tensor(
                    out=ob, in0=t_t, scalar=pb_sb[:, oc:oc + 1], in1=xr,
                    op0=OP.add, op1=OP.add)
                nc.sync.dma_start(
                    out=out_d[oc * P:(oc + 1) * P, pnch * NW:(pnch + 1) * NW], in_=ob)

            state = None
            proj_slots = {3: 0, 6: 1, 9: 2, 12: 3}
            for nch in range(NCH):
                pso_tiles = [pso_pool.tile([P, NW], F32, tag=f"pso{cs}", name=f"pso{cs}")
                             for cs in range(CCH)]
                psd_t = psd_pool.tile([P, NW], F32, tag="psd")
                prev_et = None
                prev_mt = None
                for mt in range(MT):
                    pss = pss_pool.tile([P, NW], F32, tag="pss")
                    for cq in range(CCH):
                        nc.tensor.matmul(
                            pss,
                            lhsT=u_sb[cq][:, mt * P:(mt + 1) * P],
                            rhs=r_sb[cq][:, nch * NW:(nch + 1) * NW],
                            start=(cq == 0), stop=(cq == CCH - 1))
                    et_t = et_pool.tile([P, NW], BF16, tag="et")
                    if use_amt:
                        nc.scalar.activation(out=et_t, in_=pss, func=AF.Exp,
                                             scale=INV_SQRT_C, bias=amt_sb[:, mt:mt + 1])
                    else:
                        nc.scalar.activation(out=et_t, in_=pss, func=AF.Exp,
                                             scale=INV_SQRT_C)
                    if state is not None and mt in proj_slots:
                        emit_proj(state, proj_slots[mt])
                    if prev_et is not None:
                        emit_dpv(prev_et, prev_mt, psd_t, pso_tiles,
                                 first=(prev_mt == 0), last=False)
                    prev_et = et_t
                    prev_mt = mt
                emit_dpv(prev_et, prev_mt, psd_t, pso_tiles, first=False, last=True)
                osb_list = []
                for cs in range(CCH):
                    osb = osb_pool.tile([P, NW], BF16, tag="osb")
                    nc.scalar.activation(out=osb, in_=pso_tiles[cs], func=AF.Copy)
                    osb_list.append(osb)
                rd_t = rd_pool.tile([P, NW], F32, tag="rd")
                nc.vector.reciprocal(out=rd_t, in_=psd_t)
                state = (osb_list, rd_t, nch)
            for oc in range(CCH):
                emit_proj(state, oc)

        for pool in (outsb_pool, xres_pool, tmp_pool, rd_pool, osb_pool, et_pool,
                     vt_pool, u_pool, r_pool, const):
            pool.release()

    nc.compile()
    return nc


def _prep_inputs(x, gn_gamma, gn_beta, q_w, q_b, k_w, k_b, proj_w, proj_b):
    use_kb = bool(np.any(k_b != 0))
    use_amt = bool(np.any(q_b != 0))

    bf = ml_dtypes.bfloat16
    ws_t = np.ascontiguousarray((k_w.T.astype(np.float64) @ q_w.astype(np.float64))
                                .astype(np.float32).astype(bf))
    kwt = np.ascontiguousarray(k_w.T.astype(bf))
    pwt = np.ascontiguousarray(proj_w.T.astype(bf))
    gamma_r = np.ascontiguousarray(gn_gamma.reshape(CCH, P).T.astype(np.float32))
    beta_r = np.ascontiguousarray(gn_beta.reshape(CCH, P).T.astype(np.float32))
    pb_r = np.ascontiguousarray(proj_b.reshape(CCH, P).T.astype(np.float32))
    ones_b = np.ones((P, P), dtype=bf)
    gmat = np.zeros((P, 8), dtype=np.float32)
    gmat[np.arange(P), np.arange(P) // GSIZE] = 1.0
    g2mat = np.ascontiguousarray(gmat.T)

    common = {
        "wst": ws_t, "kwt": kwt, "pwt": pwt,
        "gamma_r": gamma_r, "beta_r": beta_r, "pb_r": pb_r,
        "ones_b": ones_b, "gmat": gmat, "g2mat": g2mat,
    }
    if use_kb:
        common["kbb"] = np.ascontiguousarray(
            np.broadcast_to(k_b.astype(np.float32), (P, NW)))
    if use_amt:
        common["qbb"] = np.ascontiguousarray(
            np.broadcast_to(q_b.astype(np.float32), (P, NW)))

    in_maps = []
    for i in range(B):
        m = dict(common)
        m["x_in"] = np.ascontiguousarray(x[i].reshape(C, N).astype(np.float32))
        in_maps.append(m)
    return in_maps, use_amt, use_kb


def kernel(x, gn_gamma, gn_beta, q_w, q_b, k_w, k_b, proj_w, proj_b, _trace=False):
    x = np.asarray(x)
    in_maps, use_amt, use_kb = _prep_inputs(
        x, np.asarray(gn_gamma), np.asarray(gn_beta), np.asarray(q_w),
        np.asarray(q_b), np.asarray(k_w), np.asarray(k_b),
        np.asarray(proj_w), np.asarray(proj_b))

    key = (use_amt, use_kb)
    if key not in _BUILD_CACHE:
        _BUILD_CACHE[key] = _build(use_amt, use_kb)
    nc = _BUILD_CACHE[key]

    res = bass_utils.run_bass_kernel_spmd(
        nc, in_maps, core_ids=list(range(B)), trace=_trace)
    out = np.stack([r["out"].reshape(C, 64, 64) for r in res.results])
    kernel.last_result = res
    return out.astype(x.dtype)


def bench(inputs, iters=6):
    """Steady-state wall-clock of the 8-core NEFF execution (jit built once).

    Returns (best_ns, all_ns, outputs_list) where outputs_list[c]['out'] is
    core c's output from the final iteration.
    """
    import time
    import jax
    from jax.experimental.shard_map import shard_map
    from jax.sharding import Mesh, PartitionSpec
    from concourse import bass2jax
    import concourse.mybir as mb

    in_maps, use_amt, use_kb = _prep_inputs(
        np.asarray(inputs["x"]), np.asarray(inputs["gn_gamma"]),
        np.asarray(inputs["gn_beta"]), np.asarray(inputs["q_w"]),
        np.asarray(inputs["q_b"]), np.asarray(inputs["k_w"]),
        np.asarray(inputs["k_b"]), np.asarray(inputs["proj_w"]),
        np.asarray(inputs["proj_b"]))
    key = (use_amt, use_kb)
    if key not in _BUILD_CACHE:
        _BUILD_CACHE[key] = _build(use_amt, use_kb)
    nc = _BUILD_CACHE[key]

    bass2jax.install_neuronx_cc_hook()
    partition_name = nc.partition_id_tensor.name if nc.partition_id_tensor else None
    in_names, out_names, out_avals, zero_outs = [], [], [], []
    for alloc in nc.m.functions[0].allocations:
        if not isinstance(alloc, mb.MemoryLocationSet):
            continue
        name = alloc.memorylocations[0].name
        if alloc.kind == "ExternalInput":
            if name != partition_name:
                in_names.append(name)
        elif alloc.kind == "ExternalOutput":
            out_names.append(name)
            shape = tuple(alloc.tensor_shape)
            dtype = mb.dt.np(alloc.dtype)
            out_avals.append(jax.core.ShapedArray(shape, dtype))
            zero_outs.append(np.zeros(shape, dtype))
    n_params = len(in_names)
    n_outs = len(out_avals)
    all_names = in_names + out_names
    if partition_name is not None:
        all_names = all_names + [partition_name]

    def _body(*args):
        operands = list(args)
        if partition_name is not None:
            operands.append(bass2jax.partition_id_tensor())
        outs = bass2jax._bass_exec_p.bind(
            *operands,
            out_avals=tuple(out_avals),
            in_names=tuple(all_names),
            out_names=tuple(out_names),
            lowering_input_output_aliases=(),
            sim_require_finite=True,
            sim_require_nnan=True,
            nc=nc,
        )
        return tuple(outs)

    donate = tuple(range(n_params, n_params + n_outs))
    devices = jax.devices()[:B]
    mesh = Mesh(np.asarray(devices), ("core",))
    sharded = jax.jit(
        shard_map(_body, mesh=mesh,
                  in_specs=(PartitionSpec("core"),) * (n_params + n_outs),
                  out_specs=(PartitionSpec("core"),) * n_outs,
                  check_rep=False),
        donate_argnums=donate, keep_unused=True)

    concat_in = [
        np.concatenate([np.asarray(in_maps[c][nm]) for c in range(B)], axis=0)
        for nm in in_names
    ]
    concat_zeros = [
        np.zeros((B * z.shape[0], *z.shape[1:]), z.dtype) for z in zero_outs
    ]
    sharding = jax.sharding.NamedSharding(mesh, PartitionSpec("core"))
    dev_in = [jax.device_put(a, sharding) for a in concat_in]

    times = []
    out_arrs = None
    for _ in range(iters):
        dev_zeros = [jax.device_put(z, sharding) for z in concat_zeros]
        for z in dev_zeros:
            z.block_until_ready()
        t0 = time.perf_counter()
        out_arrs = sharded(*dev_in, *dev_zeros)
        for o in out_arrs:
            o.block_until_ready()
        times.append((time.perf_counter() - t0) * 1e9)
    outs = [
        {nm: np.asarray(out_arrs[i]).reshape(B, *out_avals[i].shape)[c]
         for i, nm in enumerate(out_names)}
        for c in range(B)
    ]
    return min(times), times, outs
